# revision 6
# baseline (speedup 1.0000x reference)
"""Trainium2 Bass kernel for nn_AttentionContextEncoder (gnn_message_passing).

reference:
  ents = ctx.T.reshape(B, 7, 4)
  prop_emb = relu(ents @ w_prop + b_prop)                      # [B,7,128]
  diffs[b,i,j,:] = ents[b,i,:] - ents[b,j,:]
  dist = sqrt(diffs[...,0]^2 + diffs[...,1]^2)
  rel = relu(concat([diffs, dist]) @ w_rel + b_rel)            # [B,7,7,128]
  rel_emb = sum_{j != i} rel[:, i, j, :]                       # [B,7,128]
  out = concat([prop_emb, rel_emb], -1)                        # [B,7,256]

v5 design (data-parallel over 8 cores, B=2048/core):
- Math as v2..v4: host G (diff combos), R (sq-dist reduce), wp/wm/wq
  images with bias folded via a ones row; K=6 matmul per directed pair;
  drains: ACT relu->r tiles, DVE fused relu-accumulate stt chains,
  GpSimd bf16 combines/folds.
- Measured facts driving v5: PE is hard-capped at 1.2 GHz (no HAM warm)
  so a 512-col MM streams in 427ns; 4-strip tile_position rotation
  overlaps MMs nearly perfectly (4 MMs / 427ns); v4's 4x[128,1024]
  PSUM pool left the PE zero runway -> isolated 625ns MMs paced the
  kernel.
- v5: DVE-drained pairs use single-bank [128,512] quarter slots
  (pool of 4) and ACT-drained pairs use [128,1024] half slots (pool
  of 2) -> 6 PSUM units in flight, PE issues quarter/half MMs
  interleaved across strips and stays off the critical path.
- Per target: pairs strip-rotated p0..p5; p0,p2,p4 -> ACT, p1,p3,p5 ->
  DVE chains (in1 = relu(p0)); GpSimd: c = r1+r2 then acc += c (t=6
  fold on DVE to cut the tail). Props: h0 relu on ACT, h1 on DVE for
  t>=2; two props front-loaded to cover rhs6 staging latency.
- Output bf16 [2,7,128,B] per core; host concatenates -> f32 ->
  transpose.
"""
import numpy as np
import ml_dtypes
from contextlib import ExitStack

import concourse.bass as bass
import concourse.bacc as bacc
import concourse.mybir as mybir
import concourse.tile as tile
from concourse.bass_utils import run_bass_kernel_spmd

F32 = mybir.dt.float32
BF16 = mybir.dt.bfloat16
AF = mybir.ActivationFunctionType
ALU = mybir.AluOpType

NUM_ENT = 7
DIM_ENT = 4
H = 128
B_TOTAL = 16384
N_CORES = 8
B = B_TOTAL // N_CORES          # 2048 per core
HB = B // 2                     # 1024
QB = B // 4                     # 512  (one PSUM bank)

_CLS = [[] for _ in range(7)]
for i in range(NUM_ENT):
    for j in range(i + 1, NUM_ENT):
        _CLS[(i + j) % 7].append((i, j))
_STRIP_PAIRS = [_CLS[0] + _CLS[1], _CLS[2] + _CLS[3], _CLS[4] + _CLS[5], _CLS[6]]
PAIRS = [p for sp in _STRIP_PAIRS for p in sp]
STRIP_NP = [len(sp) for sp in _STRIP_PAIRS]          # [6, 6, 6, 3]
STRIP_START = [0, 6, 12, 18]
PAIR_SG = {}
for s in range(4):
    for g in range(STRIP_NP[s]):
        PAIR_SG[STRIP_START[s] + g] = (s, g)
PAIR_IDX = {PAIRS[k]: k for k in range(21)}

PROP_SG = {t: (t % 4, t // 4) for t in range(NUM_ENT)}
PROP_DVE_H1 = {2, 3, 4, 5, 6}    # prop h1 drained by DVE for these targets
FOLD_DVE = {6}                   # final fold on DVE for these targets


def _ordered_pairs(t):
    by_strip = [[] for _ in range(4)]
    for j in range(NUM_ENT):
        if j == t:
            continue
        a, b_ = (t, j) if t < j else (j, t)
        s, _ = PAIR_SG[PAIR_IDX[(a, b_)]]
        by_strip[s].append(j)
    order = []
    r = 0
    while len(order) < 6:
        for s in range(4):
            if len(by_strip[s]) > r:
                order.append(by_strip[s][r])
        r += 1
    return order


def build_constants(w_prop, b_prop, w_rel, b_rel):
    bf = ml_dtypes.bfloat16
    G = np.zeros((NUM_ENT * DIM_ENT, 84), np.float32)
    for k, (i, j) in enumerate(PAIRS):
        for c in range(DIM_ENT):
            G[4 * i + c, 21 * c + k] = 1.0
            G[4 * j + c, 21 * c + k] = -1.0
    R = np.zeros((42, 21), np.float32)
    for k in range(21):
        R[k, k] = 1.0
        R[21 + k, k] = 1.0
    wp = np.zeros((H, H), np.float32)
    wm = np.zeros((H, H), np.float32)
    wq = np.zeros((H, H), np.float32)
    for s in range(4):
        r0 = 32 * s
        wp[r0:r0 + 4, :] = w_rel[0:4]
        wp[r0 + 4, :] = w_rel[4]
        wp[r0 + 5, :] = b_rel
        wm[r0:r0 + 4, :] = -w_rel[0:4]
        wm[r0 + 4, :] = w_rel[4]
        wm[r0 + 5, :] = b_rel
        wq[r0:r0 + 4, :] = w_prop
        wq[r0 + 4, :] = b_prop
    ones6 = np.ones((6, B), np.float32)
    return {
        "gmat": G.astype(bf), "rmat": R.astype(bf),
        "wpimg": wp.astype(bf), "wmimg": wm.astype(bf), "wqimg": wq.astype(bf),
        "onesb": ones6.astype(bf),
    }


def build():
    nc = bacc.Bacc("TRN2", target_bir_lowering=False, debug=False,
                   num_devices=N_CORES)
    ctxb_d = nc.dram_tensor("ctxb", [NUM_ENT * DIM_ENT, B], BF16,
                            kind="ExternalInput").ap()
    gmat_d = nc.dram_tensor("gmat", [NUM_ENT * DIM_ENT, 84], BF16,
                            kind="ExternalInput").ap()
    rmat_d = nc.dram_tensor("rmat", [42, 21], BF16, kind="ExternalInput").ap()
    wp_d = nc.dram_tensor("wpimg", [H, H], BF16, kind="ExternalInput").ap()
    wm_d = nc.dram_tensor("wmimg", [H, H], BF16, kind="ExternalInput").ap()
    wq_d = nc.dram_tensor("wqimg", [H, H], BF16, kind="ExternalInput").ap()
    ones_d = nc.dram_tensor("onesb", [6, B], BF16, kind="ExternalInput").ap()
    out_d = nc.dram_tensor("out", [2, NUM_ENT, H, B], BF16,
                           kind="ExternalOutput").ap()

    with tile.TileContext(nc) as tc, ExitStack() as ctx:
        stat = ctx.enter_context(tc.tile_pool(name="stat", bufs=1))
        rp = ctx.enter_context(tc.tile_pool(name="rp", bufs=9))
        accp = ctx.enter_context(tc.tile_pool(name="accp", bufs=3))
        poutp = ctx.enter_context(tc.tile_pool(name="poutp", bufs=4))
        cbp = ctx.enter_context(tc.tile_pool(name="cbp", bufs=3))

        # quarter slots (1 bank) for DVE-drained pairs; half slots (2
        # banks) for ACT pairs / props / prep.  4*1 + 2*2 = 8 banks.
        pslq = ctx.enter_context(tc.tile_pool(name="pslq", bufs=4,
                                              space="PSUM"))
        pslh = ctx.enter_context(tc.tile_pool(name="pslh", bufs=2,
                                              space="PSUM"))

        # ---------- inputs ----------
        ctxb = stat.tile([NUM_ENT * DIM_ENT, B], BF16)
        nc.sync.dma_start(ctxb[:], ctxb_d[:])
        gm = stat.tile([NUM_ENT * DIM_ENT, 84], BF16)
        nc.sync.dma_start(gm[:], gmat_d[:])
        wqimg = stat.tile([H, H], BF16)
        nc.sync.dma_start(wqimg[:], wq_d[:])
        onesb = stat.tile([6, B], BF16)
        nc.gpsimd.dma_start(onesb[:], ones_d[:])
        rm = stat.tile([42, 21], BF16)
        nc.gpsimd.dma_start(rm[:], rmat_d[:])
        wpimg = stat.tile([H, H], BF16)
        nc.gpsimd.dma_start(wpimg[:], wp_d[:])
        wmimg = stat.tile([H, H], BF16)
        nc.gpsimd.dma_start(wmimg[:], wm_d[:])

        prop6 = stat.tile([H, 2, B], BF16)
        for t in range(NUM_ENT):
            ps_, pg_ = PROP_SG[t]
            eng = nc.sync if t % 2 == 0 else nc.gpsimd
            eng.dma_start(prop6[32 * ps_:32 * ps_ + 4, pg_, :],
                          ctxb_d[4 * t:4 * t + 4, :])
        for s in range(4):
            nc.gpsimd.dma_start(prop6[32 * s + 4:32 * s + 5, 0:2, :],
                                onesb[0:2, :])

        # ---------- prep: G -> cmpb ----------
        cmpb = stat.tile([84, B], BF16)
        for h in range(2):
            cslot = pslh.tile([84, HB], F32, tag="hs", name="cslot")
            for c in range(2):
                nc.tensor.matmul(cslot[:, 512 * c:512 * c + 512],
                                 gm[:, :],
                                 ctxb[:, HB * h + 512 * c:HB * h + 512 * c + 512],
                                 start=True, stop=True, tile_position=(0, 0))
            nc.scalar.copy(cmpb[:, HB * h:HB * h + HB], cslot[:])

        # ---------- prop emission helper ----------
        def emit_prop(t):
            ps_, pg_ = PROP_SG[t]
            pout = poutp.tile([H, B], BF16, tag="pout", name="pout")
            for h in range(2):
                slot = pslh.tile([H, HB], F32, tag="hs", name="pslot")
                for c in range(2):
                    nc.tensor.matmul(
                        slot[:, 512 * c:512 * c + 512],
                        wqimg[32 * ps_:32 * ps_ + 5, :],
                        prop6[32 * ps_:32 * ps_ + 5, pg_,
                              HB * h + 512 * c:HB * h + 512 * c + 512],
                        start=True, stop=True, tile_position=(32 * ps_, 0))
                if h == 1 and t in PROP_DVE_H1:
                    nc.vector.tensor_single_scalar(
                        pout[:, HB * h:HB * h + HB], slot[:], 0.0, op=ALU.max)
                else:
                    nc.scalar.activation(pout[:, HB * h:HB * h + HB],
                                         slot[:], AF.Relu)
            nc.sync.dma_start(out_d[0, t, :, :], pout[:])

        # two props up front: cover rhs6 staging latency
        emit_prop(0)
        emit_prop(1)

        # sq -> R -> dist
        sq = stat.tile([42, B], BF16)
        nc.vector.tensor_mul(sq[0:42, :], cmpb[0:42, :], cmpb[0:42, :])
        distb = stat.tile([21, B], BF16)
        for h in range(2):
            dslot = pslh.tile([21, HB], F32, tag="hs", name="dslot")
            for c in range(2):
                nc.tensor.matmul(dslot[:, 512 * c:512 * c + 512],
                                 rm[0:42, :],
                                 sq[0:42, HB * h + 512 * c:HB * h + 512 * c + 512],
                                 start=True, stop=True, tile_position=(0, 0))
            nc.scalar.activation(distb[:, HB * h:HB * h + HB], dslot[:],
                                 AF.Sqrt)

        # ---------- rhs6 staging (issue spread over 3 DMA queues) ----------
        rhs6 = stat.tile([H, 6, B], BF16)
        qs = [nc.sync, nc.scalar, nc.gpsimd]
        qi = 0
        for s in range(4):
            k0, np_ = STRIP_START[s], STRIP_NP[s]
            for c in range(DIM_ENT):
                qs[qi % 3].dma_start(rhs6[32 * s + c:32 * s + c + 1, 0:np_, :],
                                     cmpb[21 * c + k0:21 * c + k0 + np_, :])
                qi += 1
            qs[qi % 3].dma_start(rhs6[32 * s + 4:32 * s + 5, 0:np_, :],
                                 distb[k0:k0 + np_, :])
            qi += 1
            qs[qi % 3].dma_start(rhs6[32 * s + 5:32 * s + 6, 0:6, :],
                                 onesb[0:6, :])
            qi += 1

        # ---------- pair groups ----------
        def pair_mm(t, j, slot, col0, ncols, tp_only=False):
            a, b_ = (t, j) if t < j else (j, t)
            s, g = PAIR_SG[PAIR_IDX[(a, b_)]]
            img = wpimg if t < j else wmimg
            nc.tensor.matmul(
                slot, img[32 * s:32 * s + 6, :],
                rhs6[32 * s:32 * s + 6, g, col0:col0 + ncols],
                start=True, stop=True, tile_position=(32 * s, 0))

        for t in range(NUM_ENT):
            order = _ordered_pairs(t)
            acc = accp.tile([H, B], BF16, tag="acc", name="acc")
            rb = []
            for w in range(3):
                pa, pd = order[2 * w], order[2 * w + 1]
                r = rp.tile([H, B], BF16, tag="r", name="r")
                rb.append(r)
                # MMs: quarters of pd interleaved with halves of pa
                qslots = []
                hslots = []
                for half in range(2):
                    hs = pslh.tile([H, HB], F32, tag="hs", name="aslot")
                    hslots.append(hs)
                    for c in range(2):
                        qslot = pslq.tile([H, QB], F32, tag="qs", name="qslot")
                        qslots.append(qslot)
                        pair_mm(t, pd, qslot[:, :],
                                512 * (2 * half + c), QB)
                        pair_mm(t, pa, hs[:, 512 * c:512 * c + 512],
                                HB * half + 512 * c, 512)
                    # drains for this half
                    nc.scalar.activation(r[:, HB * half:HB * half + HB],
                                         hslots[half][:], AF.Relu)
                    for c in range(2):
                        qc = 2 * half + c
                        in1 = (rb[0][:, QB * qc:QB * qc + QB] if w == 0
                               else acc[:, QB * qc:QB * qc + QB])
                        nc.vector.scalar_tensor_tensor(
                            acc[:, QB * qc:QB * qc + QB], qslots[qc][:],
                            0.0, in1, op0=ALU.max, op1=ALU.add)
            # combine + fold
            cb = cbp.tile([H, B], BF16, tag="c1", name="c1")
            nc.gpsimd.tensor_add(cb[:], rb[1][:], rb[2][:])
            if t in FOLD_DVE:
                nc.vector.tensor_add(acc[:], acc[:], cb[:])
            else:
                nc.gpsimd.tensor_add(acc[:], acc[:], cb[:])
            nc.sync.dma_start(out_d[1, t, :, :], acc[:])
            # spread remaining props
            if t < 5:
                emit_prop(t + 2)

    nc.compile()
    return nc


_NC_CACHE = None


def _get_nc():
    global _NC_CACHE
    if _NC_CACHE is None:
        _NC_CACHE = build()
    return _NC_CACHE


def run(ctx, w_prop, b_prop, w_rel, b_rel, trace=False):
    bf = ml_dtypes.bfloat16
    ctx = np.asarray(ctx, dtype=np.float32)
    nc = _get_nc()
    shared = build_constants(np.asarray(w_prop, np.float32),
                             np.asarray(b_prop, np.float32),
                             np.asarray(w_rel, np.float32),
                             np.asarray(b_rel, np.float32))
    in_maps = []
    for c in range(N_CORES):
        m = dict(shared)
        m["ctxb"] = np.ascontiguousarray(ctx[:, c * B:(c + 1) * B]).astype(bf)
        in_maps.append(m)
    res = run_bass_kernel_spmd(nc, in_maps, core_ids=list(range(N_CORES)),
                               trace=trace)
    shards = [np.asarray(res.results[c]["out"]).astype(np.float32)
              for c in range(N_CORES)]
    full = np.concatenate(shards, axis=3)                     # [2,7,128,16384]
    out = np.transpose(full, (3, 1, 0, 2)).reshape(B_TOTAL, NUM_ENT, 2 * H)
    return np.ascontiguousarray(out), res


def kernel(ctx, w_prop, b_prop, w_rel, b_rel):
    return run(ctx, w_prop, b_prop, w_rel, b_rel)[0]
